# revision 1
# baseline (speedup 1.0000x reference)
"""Causal self-attention (RoPE, 16 heads, B=2 T=2048 C=1024) on 8 TRN2 cores.

Sharding: core = b*4 + g  (b = batch, g = head-group of 4 heads).
Each core computes the qkv projection for its 4 heads, RoPE, causal flash
attention, and the w_proj partial product for its head slice; the host sums
the 4 partials per batch.

Matmul dtypes are configurable per stage via DTCFG (qkv, scores, y, proj):
'r' = float32r (~2 PE cycles/row, ~1.5e-4 rel err), 'b' = bfloat16
(1 cycle/row, ~2e-3 rel err). Attention processes head pairs interleaved so
the two K=64 score matmuls occupy disjoint PE row-groups and overlap.
"""

import numpy as np

# Problem constants (hardcoded per harness contract).
B = 2
T = 2048
C = 1024
N_HEAD = 16
HD = 64
HPC = 4           # heads per core
N_CORES = 8
ROPE_BASE = 10000.0
TS = 512          # qkv t-slice width
VW = HD + 1       # v_ext per-head width (v + ones column)

DTCFG = "bbbb"    # (qkv, scores, y, proj): 'r' = float32r, 'b' = bfloat16

_CACHE = {}


def _chunks512(off, end):
    """Split [off, end) on the 512 grid (PSUM bank alignment)."""
    out = []
    lo = off
    while lo < end:
        hi = min(end, (lo // 512 + 1) * 512)
        out.append((lo, hi))
        lo = hi
    return out


def _np_dt(ch):
    if ch == "b":
        import ml_dtypes
        return np.dtype(ml_dtypes.bfloat16)
    return np.dtype(np.float32)


def _build(t_len=T, dtcfg=None, debug=False):
    import concourse.tile as tile
    from concourse import bacc, mybir

    dtcfg = dtcfg or DTCFG
    F32 = mybir.dt.float32
    F32R = mybir.dt.float32r
    BF16 = mybir.dt.bfloat16

    def _dt(ch):
        return BF16 if ch == "b" else F32R

    D_QKV, D_S, D_Y, D_P = (_dt(c) for c in dtcfg)
    resident_x = dtcfg[0] == "b"   # bf16 xT (4 MB) fits in SBUF whole

    n_ts = t_len // TS          # qkv t-slices
    n_tt = t_len // 128         # 128-row t-tiles
    n_j = t_len // 1024         # attention 1024-wide tq slices

    nc = bacc.Bacc(None, target_bir_lowering=False, debug=False)
    with tile.TileContext(nc) as tc:
        with tc.tile_pool(name="dram", bufs=1, space="DRAM") as dram:
            xT = dram.tile([C, t_len], D_QKV, kind="ExternalInput")
            wqk = dram.tile([C, 8 * HD], D_QKV, kind="ExternalInput")
            wv = dram.tile([C, 4 * HD], D_QKV, kind="ExternalInput")
            wo = dram.tile([4 * HD, C], D_P, kind="ExternalInput")
            cost = dram.tile([128, t_len], F32, kind="ExternalInput")
            ssin = dram.tile([128, t_len], F32, kind="ExternalInput")
            utri = dram.tile([128, 128], D_Y, kind="ExternalInput")
            ones4 = dram.tile([128, (t_len // 128) * HPC], D_Y,
                              kind="ExternalInput")
            out = dram.tile([t_len, C], F32, kind="ExternalOutput")
            dbg = {}
            if debug:
                for m in range(4):
                    dbg[f"qkT{m}"] = dram.tile([128, t_len], F32,
                                               kind="ExternalOutput",
                                               name=f"dbg_qkT{m}")
                for k in range(2):
                    dbg[f"yT{k}"] = dram.tile([128, t_len], F32,
                                              kind="ExternalOutput",
                                              name=f"dbg_yT{k}")

            xT_c = xT.rearrange("(a p) t -> a p t", p=128)    # [8, 128, T]
            wqk_c = wqk.rearrange("(a p) m -> a p m", p=128)  # [8, 128, 512]
            wv_c = wv.rearrange("(a p) m -> a p m", p=128)    # [8, 128, 256]
            wo_c = wo.rearrange("(a p) m -> a p m", p=128)    # [2, 128, 1024]

            with (
                tc.tile_pool(name="persist", bufs=1) as persist,
                tc.tile_pool(name="qkT_pool", bufs=1) as qkT_pool,
            ):
                # Persistent tiles
                utri_sb = persist.tile([128, 128], D_Y)
                qkT = [qkT_pool.tile([128, t_len], D_S, name=f"qkT{m}")
                       for m in range(4)]
                # v_ext layout [128, n_tt, HPC, VW]: per head cols 0..63 = v,
                # col 64 = ones (softmax denominator lands on PSUM partition 64)
                vext_sb = persist.tile([128, n_tt * HPC * VW], D_Y)
                vext_v = vext_sb.rearrange("p (i h d) -> p i h d", i=n_tt, d=VW)
                yT = [persist.tile([128, t_len], D_P, name=f"yT{k}")
                      for k in range(2)]

                # ---------------- qkv phase ----------------
                with (
                    tc.tile_pool(name="wq_pool", bufs=1) as wq_pool,
                    tc.tile_pool(name="tab_pool", bufs=1) as tab_pool,
                    tc.tile_pool(name="xt_pool",
                                 bufs=(1 if resident_x else 16)) as xt_pool,
                    tc.tile_pool(name="rope_pool", bufs=2) as rope_pool,
                    tc.tile_pool(name="acc_ps_pool", bufs=6,
                                 space="PSUM") as acc_ps_pool,
                ):
                    cos_sb = tab_pool.tile([128, t_len], F32)
                    ssin_sb = tab_pool.tile([128, t_len], F32)
                    wqk_sb = [wq_pool.tile([128, 8 * HD], D_QKV, name=f"wqk{c}")
                              for c in range(8)]
                    wv_sb = [wq_pool.tile([128, 4 * HD], D_QKV, name=f"wv{c}")
                             for c in range(8)]
                    # weights on the Scalar HWDGE queue (parallel to Sync, which
                    # is busy streaming xT)
                    for c in range(8):
                        nc.scalar.dma_start(out=wv_sb[c], in_=wv_c[c])
                    for c in range(8):
                        nc.scalar.dma_start(out=wqk_sb[c], in_=wqk_c[c])

                    def rope(qkps, m, t0, width):
                        """RoPE a projected q/k PSUM tile into qkT[m]."""
                        qksb = rope_pool.tile([128, TS], F32, tag="qksb",
                                              name=f"qksb_{m}_{t0}")
                        nc.scalar.copy(out=qksb[:, :width], in_=qkps[:, :width])
                        # head dims host-permuted (evens | odds): rotate-half
                        # pair swap is a 32-partition block swap
                        swap = rope_pool.tile([128, TS], F32, tag="swap",
                                              name=f"swap_{m}_{t0}")
                        for hb in (0, 64):
                            nc.sync.dma_start(
                                out=swap[hb:hb + 32, :width],
                                in_=qksb[hb + 32:hb + 64, :width])
                            nc.sync.dma_start(
                                out=swap[hb + 32:hb + 64, :width],
                                in_=qksb[hb:hb + 32, :width])
                        tmp1 = rope_pool.tile([128, TS], F32, tag="tmp1",
                                              name=f"tmp1_{m}_{t0}")
                        nc.vector.tensor_mul(tmp1[:, :width], qkps[:, :width],
                                             cos_sb[:, t0:t0 + width])
                        tmp2 = rope_pool.tile([128, TS], F32, tag="tmp2",
                                              name=f"tmp2_{m}_{t0}")
                        nc.gpsimd.tensor_mul(tmp2[:, :width], swap[:, :width],
                                             ssin_sb[:, t0:t0 + width])
                        nc.vector.tensor_add(qkT[m][:, t0:t0 + width],
                                             tmp1[:, :width], tmp2[:, :width])

                    if resident_x:
                        xT_sb = [xt_pool.tile([128, t_len], D_QKV,
                                              name=f"xTsb{c}") for c in range(8)]
                        half = t_len // 2
                        for c in range(8):
                            nc.sync.dma_start(out=xT_sb[c][:, :half],
                                              in_=xT_c[c, :, :half])
                        for c in range(8):
                            nc.sync.dma_start(out=xT_sb[c][:, half:],
                                              in_=xT_c[c, :, half:])
                        nc.sync.dma_start(out=cos_sb, in_=cost[:])
                        nc.sync.dma_start(out=ssin_sb, in_=ssin[:])
                        nc.sync.dma_start(out=utri_sb, in_=utri[:])
                        # v in natural layout: lhsT = xT chunk (stationary
                        # switches every matmul; no reuse available)
                        for i in range(n_tt):
                            vps = acc_ps_pool.tile([128, 4 * HD], F32,
                                                   tag="acc", name=f"vps_{i}")
                            for c in range(8):
                                nc.tensor.matmul(
                                    out=vps[:],
                                    lhsT=xT_sb[c][:, i * 128:(i + 1) * 128],
                                    rhs=wv_sb[c][:],
                                    start=(c == 0), stop=(c == 7),
                                )
                            nc.vector.tensor_copy(
                                out=vext_v[:, i, :, :HD],
                                in_=vps.rearrange("p (h d) -> p h d", d=HD),
                            )
                        # ones columns, one DMA (needed before first y matmul)
                        nc.gpsimd.dma_start(
                            out=vext_v[:, :, :, HD:],
                            in_=ones4[:].rearrange("p (i h o) -> p i h o",
                                                   i=n_tt, o=1),
                        )
                        # q/k: stationary w chunk streams all n_ts t-slices
                        # (weight load amortized n_ts x)
                        for m in (2, 0, 3, 1):      # k01 q01 k23 q23
                            qkps = [acc_ps_pool.tile([128, TS], F32, tag="acc",
                                                    name=f"qkps_{m}_{ts}")
                                    for ts in range(n_ts)]
                            for c in range(8):
                                for ts in range(n_ts):
                                    nc.tensor.matmul(
                                        out=qkps[ts][:],
                                        lhsT=wqk_sb[c][:, m * 128:(m + 1) * 128],
                                        rhs=xT_sb[c][:, ts * TS:(ts + 1) * TS],
                                        start=(c == 0), stop=(c == 7),
                                    )
                            for ts in range(n_ts):
                                rope(qkps[ts], m, ts * TS, TS)
                    else:
                        nc.sync.dma_start(out=cos_sb, in_=cost[:])
                        nc.sync.dma_start(out=ssin_sb, in_=ssin[:])
                        nc.sync.dma_start(out=utri_sb, in_=utri[:])
                        for ts in range(n_ts):
                            t0 = ts * TS
                            xt = [xt_pool.tile([128, TS], D_QKV, tag="xt",
                                               name=f"xt_{ts}_{c}")
                                  for c in range(8)]
                            for c in range(8):
                                nc.sync.dma_start(out=xt[c],
                                                  in_=xT_c[c, :, t0:t0 + TS])
                            for m in range(4):
                                qkps = acc_ps_pool.tile([128, TS], F32,
                                                       tag="acc",
                                                       name=f"qkps_{ts}_{m}")
                                for c in range(8):
                                    nc.tensor.matmul(
                                        out=qkps[:],
                                        lhsT=wqk_sb[c][:, m * 128:(m + 1) * 128],
                                        rhs=xt[c][:],
                                        start=(c == 0), stop=(c == 7),
                                    )
                                rope(qkps, m, t0, TS)
                            for tt2 in range(TS // 128):
                                i = (t0 // 128) + tt2
                                vps = acc_ps_pool.tile([128, 4 * HD], F32,
                                                     tag="acc", name=f"vps_{i}")
                                for c in range(8):
                                    nc.tensor.matmul(
                                        out=vps[:],
                                        lhsT=xt[c][:, tt2 * 128:(tt2 + 1) * 128],
                                        rhs=wv_sb[c][:],
                                        start=(c == 0), stop=(c == 7),
                                    )
                                nc.vector.tensor_copy(
                                    out=vext_v[:, i, :, :HD],
                                    in_=vps.rearrange("p (h d) -> p h d", d=HD),
                                )

                    if not resident_x:
                        nc.gpsimd.dma_start(
                            out=vext_v[:, :, :, HD:],
                            in_=ones4[:].rearrange("p (i h o) -> p i h o",
                                                   i=n_tt, o=1),
                        )

                # ---------------- attention + norm ----------------
                with (
                    tc.tile_pool(name="p_pool", bufs=6) as p_pool,
                    tc.tile_pool(name="n_pool", bufs=2) as n_pool,
                    tc.tile_pool(name="wo_pool", bufs=1) as wo_pool,
                ):
                    wo_sb = [wo_pool.tile([128, C], D_P, name=f"wo{k}")
                             for k in range(2)]
                    for k in range(2):
                        nc.sync.dma_start(out=wo_sb[k], in_=wo_c[k])

                    attn_ps_cm = tc.tile_pool(name="attn_ps", space="PSUM",
                                              bufs=2)
                    ps_pool = attn_ps_cm.__enter__()
                    y_pool = ps_pool

                    def norm(yps, h, j):
                        """Copy Y off PSUM (frees the bank for the next
                        group immediately), then divide rows by the softmax
                        denominator row entirely from SBUF."""
                        base = 1024 * j
                        hoff = 64 * (h % 2)
                        ycp = n_pool.tile([65, 1024], F32, tag="ycp",
                                          name=f"ycp_{h}_{j}")
                        nc.vector.tensor_copy(out=ycp, in_=yps[:])
                        strip = n_pool.tile([8, 128], F32, tag="strip",
                                            name=f"strip_{h}_{j}")
                        nc.sync.dma_start(
                            out=strip,
                            in_=ycp[64:65, :].rearrange(
                                "p (a b) -> p a b", b=128))
                        rstrip = n_pool.tile([8, 128], F32, tag="rstrip",
                                             name=f"rstrip_{h}_{j}")
                        nc.vector.reciprocal_approx_fast(out=rstrip, in_=strip)
                        rrow = n_pool.tile([1, 1024], F32, tag="rrow",
                                           name=f"rrow_{h}_{j}")
                        nc.sync.dma_start(
                            out=rrow.rearrange("p (a b) -> p a b", b=128),
                            in_=rstrip)
                        bcast = n_pool.tile([64, 1024], F32, tag="bcast",
                                            name=f"bcast_{h}_{j}")
                        nc.gpsimd.partition_broadcast(bcast[:], rrow[:])
                        nout = n_pool.tile([64, 1024], D_P, tag="nout",
                                           name=f"nout_{h}_{j}")
                        nc.vector.tensor_mul(nout, ycp[:64, :], bcast)
                        nc.sync.dma_start(
                            out=yT[h // 2][hoff:hoff + 64, base:base + 1024],
                            in_=nout,
                        )

                    # Head pairs interleaved + 1-iteration software
                    # pipeline: while ACT computes exp for head A, the PE
                    # streams scores for head B and the y matmuls of the
                    # previous iteration — no engine ping-pong stalls.
                    for hp in range(2):
                        qtile, ktile = qkT[hp], qkT[2 + hp]
                        heads = (2 * hp, 2 * hp + 1)
                        for j in range(n_j):
                            base = 1024 * j
                            n_i = 8 * j + 8
                            yps = {h: y_pool.tile([65, 1024], F32, tag="yps",
                                                  name=f"yps_{h}_{j}")
                                   for h in heads}
                            pend = {h: [] for h in heads}

                            def emit_s(h, i):
                                hoff = 64 * (h % 2)
                                c0 = max(base, 128 * i)
                                off = c0 - base
                                ch = _chunks512(off, 1024)
                                sx = ps_pool.tile([128, 1024], F32, tag="sps",
                                                  name=f"sps_{h}_{j}_{i}")
                                for (lo, hi) in ch:
                                    nc.tensor.matmul(
                                        out=sx[:, lo:hi],
                                        lhsT=ktile[hoff:hoff + 64,
                                                   128 * i:128 * (i + 1)],
                                        rhs=qtile[hoff:hoff + 64,
                                                  base + lo:base + hi],
                                        start=True, stop=True,
                                    )
                                px = p_pool.tile([128, 1024], D_Y, tag="psb",
                                                 name=f"psb_{h}_{j}_{i}")
                                nc.scalar.activation(
                                    out=px[:, off:], in_=sx[:, off:],
                                    func=mybir.ActivationFunctionType.Exp,
                                )
                                if i >= 8 * j:
                                    nc.vector.tensor_mul(
                                        px[:, off:off + 128],
                                        px[:, off:off + 128],
                                        utri_sb,
                                    )
                                pend[h].append((i, px, ch))

                            def emit_y(h):
                                i, px, ch = pend[h].pop(0)
                                # reversed: unmasked chunks first (the masked
                                # diagonal block is in the first chunk)
                                for (lo, hi) in reversed(ch):
                                    stop_i = 8 * j + (3 if lo < 512 else 7)
                                    base_v = (i * HPC + h) * VW
                                    nc.tensor.matmul(
                                        out=yps[h][:, lo:hi],
                                        lhsT=vext_sb[:, base_v:base_v + VW],
                                        rhs=px[:, lo:hi],
                                        start=(i == 0), stop=(i == stop_i),
                                    )

                            for h in heads:
                                emit_s(h, 0)
                            for i in range(1, n_i):
                                for h in heads:
                                    emit_s(h, i)
                                for h in heads:
                                    emit_y(h)
                            for h in heads:
                                emit_y(h)
                            for h in heads:
                                norm(yps[h], h, j)

                    attn_ps_cm.__exit__(None, None, None)

                    if debug:
                        for m in range(4):
                            nc.sync.dma_start(out=dbg[f"qkT{m}"][:],
                                              in_=qkT[m][:].bitcast(F32))
                        for k in range(2):
                            nc.sync.dma_start(out=dbg[f"yT{k}"][:],
                                              in_=yT[k][:].bitcast(F32))

                    # output projection (stationary yT chunk streams both
                    # 512-wide output slices)
                    with (
                        tc.tile_pool(name="osb_pool", bufs=4) as osb_pool,
                        tc.tile_pool(name="o_ps_pool", bufs=4,
                                     space="PSUM") as o_ps_pool,
                    ):
                        for tt in range(n_tt):
                            ops = [o_ps_pool.tile([128, 512], F32, tag="ops",
                                                  name=f"ops_{tt}_{cs}")
                                   for cs in range(2)]
                            for k in range(2):
                                for cs in range(2):
                                    nc.tensor.matmul(
                                        out=ops[cs][:],
                                        lhsT=yT[k][:, tt * 128:(tt + 1) * 128],
                                        rhs=wo_sb[k][:, cs * 512:(cs + 1) * 512],
                                        start=(k == 0), stop=(k == 1),
                                    )
                            for cs in range(2):
                                osb = osb_pool.tile([128, 512], F32, tag="osb",
                                                    name=f"osb_{tt}_{cs}")
                                nc.vector.tensor_copy(out=osb, in_=ops[cs][:])
                                nc.sync.dma_start(
                                    out=out[tt * 128:(tt + 1) * 128,
                                            cs * 512:(cs + 1) * 512],
                                    in_=osb,
                                )
    nc.compile()
    names = dict(
        xT=xT.name, wqk=wqk.name, wv=wv.name, wo=wo.name,
        cost=cost.name, ssin=ssin.name, utri=utri.name, ones4=ones4.name,
        out=out.name,
    )
    for k, v in dbg.items():
        names["dbg_" + k] = v.name
    return nc, names


# Head-dim permutation: evens first, odds last — turns the interleaved
# rotate-half pair swap into a contiguous 32-row block swap on device.
PERM = np.concatenate([np.arange(0, HD, 2), np.arange(1, HD, 2)])


def _host_constants(t_len=T, dtcfg=None):
    dtcfg = dtcfg or DTCFG
    inv_freq = 1.0 / (ROPE_BASE ** (np.arange(0, HD, 2, dtype=np.float64) / HD))
    t = np.arange(t_len, dtype=np.float64)
    freqs = np.outer(t, inv_freq)                      # [T, 32]
    emb = np.concatenate([freqs, freqs], axis=-1)      # [T, 64]
    cosT = np.cos(emb).T.astype(np.float32)            # [64, T]
    sinT = np.sin(emb).T.astype(np.float32)
    sgn = np.where(np.arange(HD) % 2 == 0, -1.0, 1.0).astype(np.float32)
    ssinT = sinT * sgn[:, None]
    cosP, ssinP = cosT[PERM], ssinT[PERM]
    cos128 = np.vstack([cosP, cosP]).copy()            # [128, T]
    ssin128 = np.vstack([ssinP, ssinP]).copy()
    d_y = _np_dt(dtcfg[2])
    utri = np.triu(np.ones((128, 128), dtype=np.float32)).astype(d_y)
    ones4 = np.ones((128, (t_len // 128) * HPC), dtype=d_y)
    return cos128, ssin128, utri, ones4


def _perm_heads(w):
    """Permute each head's 64 columns of w [C, HPC*HD] by PERM."""
    Cdim = w.shape[0]
    return w.reshape(Cdim, HPC, HD)[:, :, PERM].reshape(Cdim, HPC * HD)


def _core_inputs(x, w_attn, w_proj, t_len=T, dtcfg=None):
    """Build the per-core input maps (values only, keyed by logical name)."""
    dtcfg = dtcfg or DTCFG
    d_qkv, d_p = _np_dt(dtcfg[0]), _np_dt(dtcfg[3])
    cos128, ssin128, utri, ones4 = _host_constants(t_len, dtcfg)
    per_core = []
    for core in range(N_CORES):
        b, g = divmod(core, 4)
        h0 = g * HPC * HD                       # column offset of first head
        wq = _perm_heads(w_attn[:, h0:h0 + HPC * HD])
        wk = _perm_heads(w_attn[:, C + h0:C + h0 + HPC * HD]
                         * np.float32(1.0 / np.sqrt(HD)))
        wvs = w_attn[:, 2 * C + h0:2 * C + h0 + HPC * HD]
        per_core.append(dict(
            xT=np.ascontiguousarray(x[b].T).astype(d_qkv),
            wqk=np.ascontiguousarray(np.concatenate([wq, wk], axis=1)).astype(d_qkv),
            wv=np.ascontiguousarray(wvs).astype(d_qkv),
            wo=np.ascontiguousarray(w_proj[h0:h0 + HPC * HD, :]).astype(d_p),
            cost=cos128, ssin=ssin128, utri=utri, ones4=ones4,
        ))
    return per_core


def kernel(x, w_attn, w_proj):
    from concourse.bass_utils import run_bass_kernel_spmd

    x = np.asarray(x, dtype=np.float32)
    w_attn = np.asarray(w_attn, dtype=np.float32)
    w_proj = np.asarray(w_proj, dtype=np.float32)

    if "nc" not in _CACHE:
        _CACHE["nc"], _CACHE["names"] = _build(T)
    nc, names = _CACHE["nc"], _CACHE["names"]

    per_core = _core_inputs(x, w_attn, w_proj, T)
    in_maps = [{names[k]: v for k, v in m.items()} for m in per_core]
    r = run_bass_kernel_spmd(nc, in_maps, core_ids=list(range(N_CORES)))

    full = np.zeros((B, T, C), dtype=np.float64)
    for core in range(N_CORES):
        full[core // 4] += r.results[core][names["out"]].astype(np.float64)
    return full.astype(np.float32)

